# revision 35
# baseline (speedup 1.0000x reference)
"""Additive (Bahdanau-style) attention on 8 Trainium2 NeuronCores.

Math: scores[b,q,k] = Wt . tanh(u[b,k] + v[b,q]) + bt, masked softmax over k,
out = weights @ hidden.  (bt dropped: softmax is shift-invariant.)

tanh(x) on |x| <= 9.95 ~= sum_m beta_m sin(om_m x) with 5 frequencies:
3 free bases (base 0 direct-range, bases 1-2 FRAC range-reduced) plus 2
derived doubles (2*base0, 2*base1) whose feature maps are pointwise products
of the base maps (sin2 = 2 s c, cos2 = 1 - 2 s^2), built on DVE + GPSIMD.
The angle-addition identity turns the [Sq,Sk,A] tanh tensor into PE matmuls
contracting over A.  Additive q-only score terms are softmax-invariant and
dropped; the key-side bias bu is folded into the v psum via a bu x ones seed
matmul so all range reductions are bias-free.

Masking: keys are host-gathered (padded to KW=272); pad rows get zero values
AND a zero in the denominator ones-column, so no mask bias is needed at all.
Scores accumulate transposed (psT[k,q]) in three separate PSUM banks; one
strided EXP over all three banks produces the weights; the output matmul's
ones-column yields the softmax denominator for free.

Layout: all big f16 operands ship in one packed [128, 2339] staging tensor,
split into two DMAs (proj operands first, values second) so the projection
starts as early as possible.  Garbage "heater" matmuls at the head keep the
PE HAM busy so the 4/8 cold clock throttle lifts before the real matmuls.

Sharding: core c -> batch b = c//2, query half qoff = (c%2)*256 (pure SPMD).
"""

import numpy as np

import concourse.bass as bass
import concourse.tile as tile
from concourse import bacc, mybir
from concourse.bass_utils import run_bass_kernel_spmd

# ---- problem constants (hardcoded; kernel.py must be self-contained) -------
B, S, D, A = 4, 512, 256, 128
QPC = 256          # queries per core
NCORES = 8
KW = 272           # gathered-key width (max valid count 271, padded)
KC = 3             # key chunks: 128 + 128 + 16
KCHUNK = (128, 128, 16)
FW = KW + QPC      # fused v|u feature row width
PV = 0             # v cols [0:QPC]
PU = QPC           # u cols [QPC:FW]; proj writes split at psum col 512
MAGIC = float(1.5 * 2 ** 23)     # fp32 round-to-nearest magic constant
TWO_PI = float(2.0 * np.pi)
DV = D + 1                       # values get a ones column -> denominator

# ---- tanh fit: 3 bases (base0 direct) + 2 derived (2x of bases 0,1) --------
BASES = [0.2625, 0.5905163459766474, 1.7487101922680963]
DIDX = [0, 1]
BETA = [1.4285084569414555, 1.3320835468154293, 0.08774613425554048,
        -1.2112188424747714, 0.1634796761779873]
NB = len(BASES)
ND = len(DIDX)
NTERMS = 2 * (NB + ND)

# packed f16 staging layout: per-partition column offsets
PK_WVT = 0                       # [2, 128]  wvT
PK_HTQ = PK_WVT + 256            # [2, 256]  hTq
PK_WUT = PK_HTQ + 512            # [2, 128]  wuT
PK_HTK = PK_WUT + 256            # [2, 272]  hTk
PK_HV = PK_HTK + 544             # [3, 257]  gathered values | ones(valid)
PK_TOT = PK_HV + 3 * DV          # 2339 cols f16

# const-tensor column layout: [A, NCST] f32
CST_S = 0                        # cols CST_S+j: beta_j * Wt (base v-scale)
CST_N = CST_S + NB               # cols CST_N+d: -4 beta_{NB+d} * Wt
CST_A = CST_N + ND               # cols CST_A+d:  2 beta_{NB+d} * Wt
NCST = CST_A + ND

TRACE = False                    # test.py sets True for the profiled run
LAST_EXEC_NS = None


def _ensure_ntff_hook():
    """The agent image's `antenv` lacks `axon_hooks`, so the boot-time NTFF
    hook registration silently degrades.  Recreate it: install a stub module
    and wire it to the ctypes profiler in trn_agent_boot."""
    import sys, types
    if "antenv.axon_hooks" in sys.modules:
        return
    mod = types.ModuleType("antenv.axon_hooks")
    _h = [None]
    mod.set_axon_ntff_profile_hook = lambda h: _h.__setitem__(0, h)
    mod.get_axon_ntff_profile_hook = lambda: _h[0]
    import antenv
    sys.modules["antenv.axon_hooks"] = mod
    antenv.axon_hooks = mod
    try:
        from trn_agent_boot.trn_boot import _ntff_profile_via_ctypes
        mod.set_axon_ntff_profile_hook(
            _ntff_profile_via_ctypes("/opt/axon/libaxon_pjrt.so"))
    except Exception:
        pass


# ---- custom DVE op (baseline-proven): out = t - round(t), t = in0*s0 + s1 --
_FRAC_OP = None


def _frac_reference(in0, in1, s0, s1, imm2):
    f32 = np.float32
    t = (in0.astype(f32) * f32(s0) + f32(s1)).astype(f32)
    r = ((t + f32(imm2)).astype(f32) - f32(imm2)).astype(f32)
    return (t - r).astype(f32)


def _get_frac_op():
    global _FRAC_OP
    if _FRAC_OP is not None:
        return _FRAC_OP
    from concourse import dve_ops as dvo
    from concourse.dve_spec import C0, C1, C2, Spec, Src0, lower, _has_src1
    from concourse.dve_uop import DveOpSpec

    name = "FRAC_AFFINE_ATT"
    for op in dvo.OPS:
        if op.name == name:
            _FRAC_OP = op
            return op
    t = Src0 * C0 + C1
    spec = Spec(body=t - ((t + C2) - C2), reference=_frac_reference)
    op = dvo.DveOp(name, spec, subdim=False, uops_sha={})
    dvo.OPS.append(op)
    dvo.CUSTOM_DVE_SPECS[name] = spec
    dvo._SUB_OPCODE_FOR_NAME[name] = max(dvo._SUB_OPCODE_FOR_NAME.values()) + 1
    assert dvo._SUB_OPCODE_FOR_NAME[name] < 0x20
    for ver in ("v3", "v4"):
        compiled = DveOpSpec(
            name=name,
            opcode=dvo.get_dve_sub_opcode(name),
            uops=lower(spec, ver=ver),
            rd1_en=_has_src1(spec),
        )
        op.uops_sha[ver] = compiled.sha(ver)
    _FRAC_OP = op
    return op


_NC = None


def _build_program():
    frac = _get_frac_op()
    f32 = mybir.dt.float32
    f16 = mybir.dt.float16
    nc = bacc.Bacc("TRN2", target_bir_lowering=False, debug=False,
                   num_devices=NCORES)

    pk_ext = nc.dram_tensor("pk", [128, PK_TOT], f16, kind="ExternalInput").ap()
    bu_ext = nc.dram_tensor("bu16", [1, A], f16, kind="ExternalInput").ap()
    cst_ext = nc.dram_tensor("cst", [A, NCST], f32, kind="ExternalInput").ap()
    out_ext = nc.dram_tensor("out", [QPC, D], f16, kind="ExternalOutput").ap()

    P = 128
    SIN = mybir.ActivationFunctionType.Sin
    EXP = mybir.ActivationFunctionType.Exp
    ALU = mybir.AluOpType

    with tile.TileContext(nc) as tc:
        import contextlib
        with contextlib.ExitStack() as ctx:
            const = ctx.enter_context(tc.tile_pool(name="const", bufs=1))
            pp_proj = ctx.enter_context(
                tc.tile_pool(name="pp_proj", bufs=1, space="PSUM"))
            pp_sc = ctx.enter_context(
                tc.tile_pool(name="pp_sc", bufs=1, space="PSUM"))
            pp_out = ctx.enter_context(
                tc.tile_pool(name="pp_out", bufs=1, space="PSUM"))

            # ---- SBUF tiles (all live whole-kernel) ----
            pk = const.tile([P, PK_TOT], f16, name="pk")
            cst = const.tile([P, NCST], f32, name="cst")
            bu_sb = const.tile([1, A], f16, name="bu")
            ones_q = const.tile([1, QPC], f16, name="onesq")
            hs = const.tile([P, 512], f16, name="hs")
            z1 = const.tile([P, 1], f32, name="z1")
            pio2t = const.tile([P, 1], f32, name="pio2")
            scr = const.tile([P, 1], f16, name="scr")
            ft = [const.tile([P, 2, FW], f16, name=f"ft{j}")
                  for j in range(NB)]
            fr = {j: const.tile([P, 2, FW], f16, name=f"fr{j}")
                  for j in (1, 2)}
            vmc = [const.tile([P, QPC], f16, name=f"vmc{j}")
                   for j in range(NB)]
            vms = [const.tile([P, QPC], f16, name=f"vms{j}")
                   for j in range(NB)]
            du = [const.tile([P, 2, KW], f16, name=f"du{d}")
                  for d in range(ND)]
            dp0 = [const.tile([P, QPC], f16, name=f"dp0_{d}")
                   for d in range(ND)]
            dp1 = [const.tile([P, QPC], f16, name=f"dp1_{d}")
                   for d in range(ND)]
            dva = [const.tile([P, QPC], f16, name=f"dva{d}")
                   for d in range(ND)]
            dvb = [const.tile([P, QPC], f16, name=f"dvb{d}")
                   for d in range(ND)]
            expw = const.tile([P, KC, QPC], f16, name="expw")
            osb = [const.tile([P, D], f16, name=f"osb{qt}") for qt in range(2)]
            rsum = [const.tile([P, 1], f32, name=f"rs{qt}") for qt in range(2)]

            # ---- PSUM tiles ----
            # v and u projections in SEPARATE banks so ACT (direct SINs)
            # and DVE (FRACs) can read them concurrently.
            pv = pp_proj.tile([P, QPC], f32, name="pv")            # bank 0
            pu = pp_proj.tile([P, KW], f32, name="pu")             # bank 1
            pT = pp_sc.tile([P, KC, 512], f32, name="pT")          # banks 2-4
            heat_ps = pp_out.tile([P, 512], f32, name="heat")      # bank 5
            ps_o = [pp_out.tile([P, DV], f32, name=f"ps_o{qt}")
                    for qt in range(2)]                            # banks 6-7

            def wvT(c):
                return pk[:, PK_WVT + c * 128:PK_WVT + (c + 1) * 128]

            def hTq(c):
                return pk[:, PK_HTQ + c * 256:PK_HTQ + (c + 1) * 256]

            def wuT(c):
                return pk[:, PK_WUT + c * 128:PK_WUT + (c + 1) * 128]

            def hTk(c):
                return pk[:, PK_HTK + c * KW:PK_HTK + (c + 1) * KW]

            def hv(c):
                return pk[:, PK_HV + c * DV:PK_HV + (c + 1) * DV]

            # ---- early constants (gpsimd queue; DVE queue stays clear) ----
            nc.gpsimd.memset(z1, 0.0)
            nc.gpsimd.memset(pio2t, float(np.pi / 2))
            nc.gpsimd.memset(ones_q, 1.0)
            nc.vector.memset(hs, 0.5)

            # ---- input DMAs: one big-descriptor transfer for proj data ----
            nc.gpsimd.dma_start(out=pk[:, 0:PK_HV], in_=pk_ext[:, 0:PK_HV])
            nc.sync.dma_start(out=pk[:, PK_HV:PK_TOT],
                              in_=pk_ext[:, PK_HV:PK_TOT])
            nc.scalar.dma_start(out=bu_sb, in_=bu_ext[:])
            nc.scalar.dma_start(out=cst, in_=cst_ext[:])

            # warm the trig table immediately (z1 is memset, not DMA-gated)
            nc.scalar.activation(scr, z1, SIN, bias=z1, scale=1.0)

            # ---- PE clock heaters: garbage matmuls lift the 4/8 throttle --
            def heat(n, width=512):
                for _ in range(n):
                    nc.tensor.matmul(heat_ps[:, 0:width], hs[:, 0:P],
                                     hs[:, 0:width], start=True, stop=True,
                                     skip_group_check=True)

            heat(6)

            # ---- projections: v-group -> pv (bank 0), u-group -> pu ----
            nc.tensor.matmul(pv, wvT(0), hTq(0), start=True, stop=False)
            nc.tensor.matmul(pv, wvT(1), hTq(1), start=False, stop=False)
            nc.tensor.matmul(pv, bu_sb, ones_q, start=False, stop=True)
            for c in range(2):
                nc.tensor.matmul(pu, wuT(c), hTk(c),
                                 start=(c == 0), stop=(c == 1))

            heat(5)

            # ---- range reductions (DVE) read pv/pu banks directly; the
            # direct-base SINs (ACT) interleave on the opposite bank.
            def emit_frac_u(j):
                s0 = float(BASES[j] / TWO_PI)
                nc.vector._custom_dve(frac, out=fr[j][:, 0, PU:FW], in0=pu,
                                      s0=s0, s1=0.0, imm2=MAGIC)
                nc.vector._custom_dve(frac, out=fr[j][:, 1, PU:FW], in0=pu,
                                      s0=s0, s1=0.25, imm2=MAGIC)

            def emit_frac_v(j):
                s0 = float(BASES[j] / TWO_PI)
                nc.vector._custom_dve(frac, out=fr[j][:, 0, PV:PV + QPC],
                                      in0=pv, s0=s0, s1=0.0, imm2=MAGIC)
                nc.vector._custom_dve(frac, out=fr[j][:, 1, PV:PV + QPC],
                                      in0=pv, s0=s0, s1=0.25, imm2=MAGIC)

            # ---- direct-base sines straight from psum (ACT), v then u ----
            w0 = float(BASES[0])
            nc.scalar.activation(ft[0][:, 0, PV:PV + QPC], pv, SIN,
                                 bias=z1, scale=w0)
            nc.scalar.activation(ft[0][:, 1, PV:PV + QPC], pv, SIN,
                                 bias=pio2t, scale=w0)

            # ---- v-side scales + derived maps ----
            def emit_vscale(j, eng=None):
                eng = eng or nc.vector
                sc_col = cst[:, CST_S + j:CST_S + j + 1]
                eng.tensor_scalar(
                    out=vmc[j], in0=ft[j][:, 1, PV:PV + QPC],
                    scalar1=sc_col, scalar2=None, op0=ALU.mult)
                eng.tensor_scalar(
                    out=vms[j], in0=ft[j][:, 0, PV:PV + QPC],
                    scalar1=sc_col, scalar2=None, op0=ALU.mult)

            def emit_derived_u(d):
                # GPSIMD: du pages (s_u^2, s_u c_u) from parent maps
                p = DIDX[d]
                su = ft[p][:, 0, PU:FW]
                cu = ft[p][:, 1, PU:FW]
                nc.gpsimd.tensor_tensor(out=du[d][:, 0, :], in0=su, in1=su,
                                        op=ALU.mult)
                nc.gpsimd.tensor_tensor(out=du[d][:, 1, :], in0=su, in1=cu,
                                        op=ALU.mult)

            def emit_derived_v(d):
                # dp0 = S_p*Wt*sv^2, dp1 = S_p*Wt*sv*cv (plain TT from vms);
                # va = cdiv*dp0 + 2 b_d Wt;  vb = cdiv*dp1,  cdiv=-4 b_d/b_p
                p = DIDX[d]
                sv = ft[p][:, 0, PV:PV + QPC]
                cv = ft[p][:, 1, PV:PV + QPC]
                cdiv = float(-4.0 * BETA[NB + d] / BETA[p])
                aCol = cst[:, CST_A + d:CST_A + d + 1]
                nc.vector.tensor_tensor(out=dp0[d], in0=vms[p], in1=sv,
                                        op=ALU.mult)
                nc.vector.tensor_tensor(out=dp1[d], in0=vms[p], in1=cv,
                                        op=ALU.mult)
                nc.vector.tensor_scalar(
                    out=dva[d], in0=dp0[d], scalar1=cdiv, scalar2=None,
                    op0=ALU.mult)
                nc.vector.tensor_scalar(
                    out=dva[d], in0=dva[d], scalar1=aCol, scalar2=None,
                    op0=ALU.add)
                nc.vector.tensor_scalar(
                    out=dvb[d], in0=dp1[d], scalar1=cdiv, scalar2=None,
                    op0=ALU.mult)

            emit_frac_u(1)
            nc.scalar.activation(ft[0][:, 0, PU:FW], pu, SIN,
                                 bias=z1, scale=w0)
            nc.scalar.activation(ft[0][:, 1, PU:FW], pu, SIN,
                                 bias=pio2t, scale=w0)
            emit_vscale(0)
            emit_frac_v(1)
            nc.scalar.activation(ft[1], fr[1], SIN, bias=z1, scale=TWO_PI)
            emit_derived_v(0)
            emit_derived_u(0)
            emit_frac_u(2)
            emit_frac_v(2)
            nc.scalar.activation(ft[2], fr[2], SIN, bias=z1, scale=TWO_PI)
            emit_vscale(1)
            emit_derived_u(1)
            emit_derived_v(1)
            emit_vscale(2)

            # ---- score matmuls, transposed psT[k,q], term-major ----------
            # term list in production order: (u_tile, page, rhs)
            terms = [
                (ft[0], 0, vmc[0]), (ft[0], 1, vms[0]),
                (du[0], 1, dva[0]), (du[0], 0, dvb[0]),
                (ft[1], 0, vmc[1]), (ft[1], 1, vms[1]),
                (du[1], 1, dva[1]), (du[1], 0, dvb[1]),
                (ft[2], 0, vmc[2]), (ft[2], 1, vms[2]),
            ]
            assert len(terms) == NTERMS
            for t, (ut, pg, rhs) in enumerate(terms):
                uoff = PU if ut.shape[2] == FW else 0
                for c in range(KC):
                    pc = KCHUNK[c]
                    nc.tensor.matmul(
                        pT[0:pc, c, 0:QPC],
                        ut[:, pg, uoff + c * 128:uoff + c * 128 + pc],
                        rhs, start=(t == 0), stop=(t == NTERMS - 1))

            # ---- softmax weights: strided EXP over all 3 psT banks, split
            # per query half so qt0's output matmuls overlap qt1's EXP.
            for qt in range(2):
                qs = slice(qt * P, (qt + 1) * P)
                nc.scalar.activation(expw[:, :, qs], pT[:, :, qs], EXP,
                                     bias=z1, scale=1.0)

            # ---- output: out[q,d] = sum_k expw[k,q] hv[k,d] --------------
            for qt in range(2):
                qs = slice(qt * P, (qt + 1) * P)
                for c in range(KC):
                    pc = KCHUNK[c]
                    nc.tensor.matmul(ps_o[qt], expw[0:pc, c, qs],
                                     hv(c)[0:pc, :],
                                     start=(c == 0), stop=(c == KC - 1))
                nc.vector.reciprocal(rsum[qt], ps_o[qt][:, D:DV])
                nc.vector.tensor_scalar(out=osb[qt], in0=ps_o[qt][:, 0:D],
                                        scalar1=rsum[qt],
                                        scalar2=None, op0=ALU.mult)
                if qt == 0:
                    nc.sync.dma_start(out=out_ext[qs, :], in_=osb[qt])
                else:
                    nc.scalar.dma_start(out=out_ext[qs, :], in_=osb[qt])

    nc.compile()
    return nc


def _make_cst(Wt_f):
    cst = np.zeros((A, NCST), dtype=np.float32)
    for j in range(NB):
        cst[:, CST_S + j] = BETA[j] * Wt_f
    for d in range(ND):
        bd = BETA[NB + d]
        cst[:, CST_N + d] = -4.0 * bd * Wt_f
        cst[:, CST_A + d] = 2.0 * bd * Wt_f
    return cst


def _chunk_rows(a, nrow):
    """[nrow*128, W] -> per-partition packed [128, nrow*W]."""
    W = a.shape[1]
    return np.ascontiguousarray(
        a.reshape(nrow, 128, W).transpose(1, 0, 2).reshape(128, nrow * W))


def _pack_core(hq, htk, hv_pad, WuT16, WvT16):
    cols = [
        _chunk_rows(WvT16, 2),             # [128, 256]
        _chunk_rows(hq.T, 2),              # [128, 512]
        _chunk_rows(WuT16, 2),             # [128, 256]
        _chunk_rows(htk, 2),               # [128, 544]
        _chunk_rows(hv_pad, 3),            # [128, 3*DV]
    ]
    return np.ascontiguousarray(np.concatenate(cols, axis=1))


def kernel(hidden, mask, Wu, bu, Wv, Wt, bt):
    global _NC, LAST_EXEC_NS
    if _NC is None:
        _NC = _build_program()
    nc = _NC

    hidden = np.asarray(hidden, dtype=np.float32)
    mask = np.asarray(mask)
    Wu = np.asarray(Wu, dtype=np.float32)
    Wv = np.asarray(Wv, dtype=np.float32)
    Wt_f = np.asarray(Wt, dtype=np.float32).reshape(A)
    bu_f = np.asarray(bu, dtype=np.float32).reshape(A)

    WuT16 = Wu.T.astype(np.float16)        # [D, A]
    WvT16 = Wv.T.astype(np.float16)
    cst = _make_cst(Wt_f)
    bu16 = np.ascontiguousarray(bu_f.astype(np.float16).reshape(1, A))

    # per-batch gathered keys (shared by the two cores of a batch)
    batch_prep = []
    for b in range(B):
        valid = np.where(np.asarray(mask[b]) >= 1)[0]
        nv = len(valid)
        assert nv <= KW, f"valid keys {nv} > KW={KW}"
        hk = hidden[b][valid].astype(np.float16)            # [nv, D]
        htk = np.zeros((D, KW), dtype=np.float16)
        htk[:, :nv] = hk.T
        hv_pad = np.zeros((3 * 128, DV), dtype=np.float16)
        hv_pad[:nv, :D] = hk
        hv_pad[:nv, D] = 1.0               # ones only on VALID rows
        batch_prep.append((htk, hv_pad))

    in_maps = []
    for c in range(NCORES):
        b, half = divmod(c, 2)
        qoff = half * QPC
        htk, hv_pad = batch_prep[b]
        hq = hidden[b, qoff:qoff + QPC].astype(np.float16)
        pk = _pack_core(hq, htk, hv_pad, WuT16, WvT16)
        in_maps.append({"pk": pk, "bu16": bu16, "cst": cst})

    if TRACE:
        _ensure_ntff_hook()
    res = run_bass_kernel_spmd(nc, in_maps, list(range(NCORES)), trace=TRACE)
    LAST_EXEC_NS = res.exec_time_ns

    out = np.empty((B, S, D), dtype=np.float32)
    for c in range(NCORES):
        b, half = divmod(c, 2)
        qoff = half * QPC
        out[c // 2, qoff:qoff + QPC] = res.results[c]["out"].astype(np.float32)
    return out


# revision 36
# speedup vs baseline: 1.0048x; 1.0048x over previous
"""Additive (Bahdanau-style) attention on 8 Trainium2 NeuronCores.

Math: scores[b,q,k] = Wt . tanh(u[b,k] + v[b,q]) + bt, masked softmax over k,
out = weights @ hidden.  (bt dropped: softmax is shift-invariant.)

tanh(x) on |x| <= 9.95 ~= sum_m beta_m sin(om_m x) with 5 frequencies:
3 free bases (base 0 direct-range, bases 1-2 FRAC range-reduced) plus 2
derived doubles (2*base0, 2*base1) whose feature maps are pointwise products
of the base maps (sin2 = 2 s c, cos2 = 1 - 2 s^2), built on DVE + GPSIMD.
The angle-addition identity turns the [Sq,Sk,A] tanh tensor into PE matmuls
contracting over A.  Additive q-only score terms are softmax-invariant and
dropped; the key-side bias bu is folded into the v psum via a bu x ones seed
matmul so all range reductions are bias-free.

Masking: keys are host-gathered (padded to KW=272); pad rows get zero values
AND a zero in the denominator ones-column, so no mask bias is needed at all.
Scores accumulate transposed (psT[k,q]) in three separate PSUM banks; one
strided EXP over all three banks produces the weights; the output matmul's
ones-column yields the softmax denominator for free.

Layout: all big f16 operands ship in one packed [128, 2339] staging tensor,
split into two DMAs (proj operands first, values second) so the projection
starts as early as possible.  Garbage "heater" matmuls at the head keep the
PE HAM busy so the 4/8 cold clock throttle lifts before the real matmuls.

Sharding: core c -> batch b = c//2, query half qoff = (c%2)*256 (pure SPMD).
"""

import numpy as np

import concourse.bass as bass
import concourse.tile as tile
from concourse import bacc, mybir
from concourse.bass_utils import run_bass_kernel_spmd

# ---- problem constants (hardcoded; kernel.py must be self-contained) -------
B, S, D, A = 4, 512, 256, 128
QPC = 256          # queries per core
NCORES = 8
KW = 272           # gathered-key width (max valid count 271, padded)
KC = 3             # key chunks: 128 + 128 + 16
KCHUNK = (128, 128, 16)
FW = KW + QPC      # fused v|u feature row width
PV = 0             # v cols [0:QPC]
PU = QPC           # u cols [QPC:FW]; proj writes split at psum col 512
MAGIC = float(1.5 * 2 ** 23)     # fp32 round-to-nearest magic constant
TWO_PI = float(2.0 * np.pi)
DV = D + 1                       # values get a ones column -> denominator

# ---- tanh fit: 3 bases (base0 direct) + 2 derived (2x of bases 0,1) --------
BASES = [0.2625, 0.5905163459766474, 1.7487101922680963]
DIDX = [0, 1]
BETA = [1.4285084569414555, 1.3320835468154293, 0.08774613425554048,
        -1.2112188424747714, 0.1634796761779873]
NB = len(BASES)
ND = len(DIDX)
NTERMS = 2 * (NB + ND)

# packed f16 staging layout: per-partition column offsets
PK_WVT = 0                       # [2, 128]  wvT
PK_HTQ = PK_WVT + 256            # [2, 256]  hTq
PK_WUT = PK_HTQ + 512            # [2, 128]  wuT
PK_HTK = PK_WUT + 256            # [2, 272]  hTk
PK_HV = PK_HTK + 544             # [3, 257]  gathered values | ones(valid)
PK_TOT = PK_HV + 3 * DV          # 2339 cols f16

# const-tensor column layout: [A, NCST] f32
CST_S = 0                        # cols CST_S+j: beta_j * Wt (base v-scale)
CST_N = CST_S + NB               # cols CST_N+d: -4 beta_{NB+d} * Wt
CST_A = CST_N + ND               # cols CST_A+d:  2 beta_{NB+d} * Wt
NCST = CST_A + ND

TRACE = False                    # test.py sets True for the profiled run
LAST_EXEC_NS = None


def _ensure_ntff_hook():
    """The agent image's `antenv` lacks `axon_hooks`, so the boot-time NTFF
    hook registration silently degrades.  Recreate it: install a stub module
    and wire it to the ctypes profiler in trn_agent_boot."""
    import sys, types
    if "antenv.axon_hooks" in sys.modules:
        return
    mod = types.ModuleType("antenv.axon_hooks")
    _h = [None]
    mod.set_axon_ntff_profile_hook = lambda h: _h.__setitem__(0, h)
    mod.get_axon_ntff_profile_hook = lambda: _h[0]
    import antenv
    sys.modules["antenv.axon_hooks"] = mod
    antenv.axon_hooks = mod
    try:
        from trn_agent_boot.trn_boot import _ntff_profile_via_ctypes
        mod.set_axon_ntff_profile_hook(
            _ntff_profile_via_ctypes("/opt/axon/libaxon_pjrt.so"))
    except Exception:
        pass


# ---- custom DVE op (baseline-proven): out = t - round(t), t = in0*s0 + s1 --
_FRAC_OP = None


def _frac_reference(in0, in1, s0, s1, imm2):
    f32 = np.float32
    t = (in0.astype(f32) * f32(s0) + f32(s1)).astype(f32)
    r = ((t + f32(imm2)).astype(f32) - f32(imm2)).astype(f32)
    return (t - r).astype(f32)


def _get_frac_op():
    global _FRAC_OP
    if _FRAC_OP is not None:
        return _FRAC_OP
    from concourse import dve_ops as dvo
    from concourse.dve_spec import C0, C1, C2, Spec, Src0, lower, _has_src1
    from concourse.dve_uop import DveOpSpec

    name = "FRAC_AFFINE_ATT"
    for op in dvo.OPS:
        if op.name == name:
            _FRAC_OP = op
            return op
    t = Src0 * C0 + C1
    spec = Spec(body=t - ((t + C2) - C2), reference=_frac_reference)
    op = dvo.DveOp(name, spec, subdim=False, uops_sha={})
    dvo.OPS.append(op)
    dvo.CUSTOM_DVE_SPECS[name] = spec
    dvo._SUB_OPCODE_FOR_NAME[name] = max(dvo._SUB_OPCODE_FOR_NAME.values()) + 1
    assert dvo._SUB_OPCODE_FOR_NAME[name] < 0x20
    for ver in ("v3", "v4"):
        compiled = DveOpSpec(
            name=name,
            opcode=dvo.get_dve_sub_opcode(name),
            uops=lower(spec, ver=ver),
            rd1_en=_has_src1(spec),
        )
        op.uops_sha[ver] = compiled.sha(ver)
    _FRAC_OP = op
    return op


_NC = None


def _build_program():
    frac = _get_frac_op()
    f32 = mybir.dt.float32
    f16 = mybir.dt.float16
    nc = bacc.Bacc("TRN2", target_bir_lowering=False, debug=False,
                   num_devices=NCORES)

    pk_ext = nc.dram_tensor("pk", [128, PK_TOT], f16, kind="ExternalInput").ap()
    bu_ext = nc.dram_tensor("bu16", [1, A], f16, kind="ExternalInput").ap()
    cst_ext = nc.dram_tensor("cst", [A, NCST], f32, kind="ExternalInput").ap()
    out_ext = nc.dram_tensor("out", [QPC, D], f16, kind="ExternalOutput").ap()

    P = 128
    SIN = mybir.ActivationFunctionType.Sin
    EXP = mybir.ActivationFunctionType.Exp
    ALU = mybir.AluOpType

    with tile.TileContext(nc) as tc:
        import contextlib
        with contextlib.ExitStack() as ctx:
            const = ctx.enter_context(tc.tile_pool(name="const", bufs=1))
            pp_proj = ctx.enter_context(
                tc.tile_pool(name="pp_proj", bufs=1, space="PSUM"))
            pp_sc = ctx.enter_context(
                tc.tile_pool(name="pp_sc", bufs=1, space="PSUM"))
            pp_out = ctx.enter_context(
                tc.tile_pool(name="pp_out", bufs=1, space="PSUM"))

            # ---- SBUF tiles (all live whole-kernel) ----
            pk = const.tile([P, PK_TOT], f16, name="pk")
            cst = const.tile([P, NCST], f32, name="cst")
            bu_sb = const.tile([1, A], f16, name="bu")
            ones_q = const.tile([1, QPC], f16, name="onesq")
            hs = const.tile([P, 512], f16, name="hs")
            z1 = const.tile([P, 1], f32, name="z1")
            pio2t = const.tile([P, 1], f32, name="pio2")
            scr = const.tile([P, 1], f16, name="scr")
            ft = [const.tile([P, 2, FW], f16, name=f"ft{j}")
                  for j in range(NB)]
            fr = {j: const.tile([P, 2, FW], f16, name=f"fr{j}")
                  for j in (1, 2)}
            vmc = [const.tile([P, QPC], f16, name=f"vmc{j}")
                   for j in range(NB)]
            vms = [const.tile([P, QPC], f16, name=f"vms{j}")
                   for j in range(NB)]
            du = [const.tile([P, 2, KW], f16, name=f"du{d}")
                  for d in range(ND)]
            dp0 = [const.tile([P, QPC], f16, name=f"dp0_{d}")
                   for d in range(ND)]
            dp1 = [const.tile([P, QPC], f16, name=f"dp1_{d}")
                   for d in range(ND)]
            dva = [const.tile([P, QPC], f16, name=f"dva{d}")
                   for d in range(ND)]
            dvb = [const.tile([P, QPC], f16, name=f"dvb{d}")
                   for d in range(ND)]
            expw = const.tile([P, KC, QPC], f16, name="expw")
            osb = [const.tile([P, D], f16, name=f"osb{qt}") for qt in range(2)]
            rsum = [const.tile([P, 1], f32, name=f"rs{qt}") for qt in range(2)]

            # ---- PSUM tiles ----
            # v and u projections in SEPARATE banks so ACT (direct SINs)
            # and DVE (FRACs) can read them concurrently.
            pv = pp_proj.tile([P, QPC], f32, name="pv")            # bank 0
            pu = pp_proj.tile([P, KW], f32, name="pu")             # bank 1
            pT = pp_sc.tile([P, KC, 512], f32, name="pT")          # banks 2-4
            heat_ps = pp_out.tile([P, 512], f32, name="heat")      # bank 5
            ps_o = [pp_out.tile([P, DV], f32, name=f"ps_o{qt}")
                    for qt in range(2)]                            # banks 6-7

            def wvT(c):
                return pk[:, PK_WVT + c * 128:PK_WVT + (c + 1) * 128]

            def hTq(c):
                return pk[:, PK_HTQ + c * 256:PK_HTQ + (c + 1) * 256]

            def wuT(c):
                return pk[:, PK_WUT + c * 128:PK_WUT + (c + 1) * 128]

            def hTk(c):
                return pk[:, PK_HTK + c * KW:PK_HTK + (c + 1) * KW]

            def hv(c):
                return pk[:, PK_HV + c * DV:PK_HV + (c + 1) * DV]

            # ---- input DMAs first on the gpsimd queue (earliest preamble
            # exit), values behind proj data on the same queue ----
            nc.gpsimd.dma_start(out=pk[:, 0:PK_HV], in_=pk_ext[:, 0:PK_HV])
            nc.gpsimd.dma_start(out=pk[:, PK_HV:PK_TOT],
                                in_=pk_ext[:, PK_HV:PK_TOT])
            nc.scalar.dma_start(out=bu_sb, in_=bu_ext[:])
            nc.sync.dma_start(out=cst, in_=cst_ext[:])

            # ---- early constants ----
            nc.gpsimd.memset(z1, 0.0)
            nc.gpsimd.memset(pio2t, float(np.pi / 2))
            nc.gpsimd.memset(ones_q, 1.0)
            nc.vector.memset(hs, 0.5)

            # warm the trig table immediately (z1 is memset, not DMA-gated)
            nc.scalar.activation(scr, z1, SIN, bias=z1, scale=1.0)

            # ---- PE clock heaters: garbage matmuls lift the 4/8 throttle --
            def heat(n, width=512):
                for _ in range(n):
                    nc.tensor.matmul(heat_ps[:, 0:width], hs[:, 0:P],
                                     hs[:, 0:width], start=True, stop=True,
                                     skip_group_check=True)

            heat(6)

            # ---- projections: v-group -> pv (bank 0), u-group -> pu ----
            nc.tensor.matmul(pv, wvT(0), hTq(0), start=True, stop=False)
            nc.tensor.matmul(pv, wvT(1), hTq(1), start=False, stop=False)
            nc.tensor.matmul(pv, bu_sb, ones_q, start=False, stop=True)
            for c in range(2):
                nc.tensor.matmul(pu, wuT(c), hTk(c),
                                 start=(c == 0), stop=(c == 1))

            heat(5)

            # ---- range reductions (DVE) read pv/pu banks directly; the
            # direct-base SINs (ACT) interleave on the opposite bank.
            def emit_frac_u(j):
                s0 = float(BASES[j] / TWO_PI)
                nc.vector._custom_dve(frac, out=fr[j][:, 0, PU:FW], in0=pu,
                                      s0=s0, s1=0.0, imm2=MAGIC)
                nc.vector._custom_dve(frac, out=fr[j][:, 1, PU:FW], in0=pu,
                                      s0=s0, s1=0.25, imm2=MAGIC)

            def emit_frac_v(j):
                s0 = float(BASES[j] / TWO_PI)
                nc.vector._custom_dve(frac, out=fr[j][:, 0, PV:PV + QPC],
                                      in0=pv, s0=s0, s1=0.0, imm2=MAGIC)
                nc.vector._custom_dve(frac, out=fr[j][:, 1, PV:PV + QPC],
                                      in0=pv, s0=s0, s1=0.25, imm2=MAGIC)

            # ---- direct-base sines straight from psum (ACT), v then u ----
            w0 = float(BASES[0])
            nc.scalar.activation(ft[0][:, 0, PV:PV + QPC], pv, SIN,
                                 bias=z1, scale=w0)
            nc.scalar.activation(ft[0][:, 1, PV:PV + QPC], pv, SIN,
                                 bias=pio2t, scale=w0)

            # ---- v-side scales + derived maps ----
            def emit_vscale(j, eng=None):
                eng = eng or nc.vector
                sc_col = cst[:, CST_S + j:CST_S + j + 1]
                eng.tensor_scalar(
                    out=vmc[j], in0=ft[j][:, 1, PV:PV + QPC],
                    scalar1=sc_col, scalar2=None, op0=ALU.mult)
                eng.tensor_scalar(
                    out=vms[j], in0=ft[j][:, 0, PV:PV + QPC],
                    scalar1=sc_col, scalar2=None, op0=ALU.mult)

            def emit_derived_u(d):
                # GPSIMD: du pages (s_u^2, s_u c_u) from parent maps
                p = DIDX[d]
                su = ft[p][:, 0, PU:FW]
                cu = ft[p][:, 1, PU:FW]
                nc.gpsimd.tensor_tensor(out=du[d][:, 0, :], in0=su, in1=su,
                                        op=ALU.mult)
                nc.gpsimd.tensor_tensor(out=du[d][:, 1, :], in0=su, in1=cu,
                                        op=ALU.mult)

            def emit_derived_v(d):
                # dp0 = S_p*Wt*sv^2, dp1 = S_p*Wt*sv*cv (plain TT from vms);
                # va = cdiv*dp0 + 2 b_d Wt;  vb = cdiv*dp1,  cdiv=-4 b_d/b_p
                p = DIDX[d]
                sv = ft[p][:, 0, PV:PV + QPC]
                cv = ft[p][:, 1, PV:PV + QPC]
                cdiv = float(-4.0 * BETA[NB + d] / BETA[p])
                aCol = cst[:, CST_A + d:CST_A + d + 1]
                nc.vector.tensor_tensor(out=dp0[d], in0=vms[p], in1=sv,
                                        op=ALU.mult)
                nc.vector.tensor_tensor(out=dp1[d], in0=vms[p], in1=cv,
                                        op=ALU.mult)
                nc.vector.tensor_scalar(
                    out=dva[d], in0=dp0[d], scalar1=cdiv, scalar2=None,
                    op0=ALU.mult)
                nc.vector.tensor_scalar(
                    out=dva[d], in0=dva[d], scalar1=aCol, scalar2=None,
                    op0=ALU.add)
                nc.vector.tensor_scalar(
                    out=dvb[d], in0=dp1[d], scalar1=cdiv, scalar2=None,
                    op0=ALU.mult)

            emit_frac_u(1)
            nc.scalar.activation(ft[0][:, 0, PU:FW], pu, SIN,
                                 bias=z1, scale=w0)
            nc.scalar.activation(ft[0][:, 1, PU:FW], pu, SIN,
                                 bias=pio2t, scale=w0)
            emit_vscale(0)
            emit_frac_v(1)
            nc.scalar.activation(ft[1], fr[1], SIN, bias=z1, scale=TWO_PI)
            emit_derived_v(0)
            emit_derived_u(0)
            emit_frac_u(2)
            emit_frac_v(2)
            nc.scalar.activation(ft[2], fr[2], SIN, bias=z1, scale=TWO_PI)
            emit_vscale(1)
            emit_derived_u(1)
            emit_derived_v(1)
            emit_vscale(2)

            # ---- score matmuls, transposed psT[k,q], term-major ----------
            # term list in production order: (u_tile, page, rhs)
            terms = [
                (ft[0], 0, vmc[0]), (ft[0], 1, vms[0]),
                (du[0], 1, dva[0]), (du[0], 0, dvb[0]),
                (ft[1], 0, vmc[1]), (ft[1], 1, vms[1]),
                (du[1], 1, dva[1]), (du[1], 0, dvb[1]),
                (ft[2], 0, vmc[2]), (ft[2], 1, vms[2]),
            ]
            assert len(terms) == NTERMS
            for t, (ut, pg, rhs) in enumerate(terms):
                uoff = PU if ut.shape[2] == FW else 0
                for c in range(KC):
                    pc = KCHUNK[c]
                    nc.tensor.matmul(
                        pT[0:pc, c, 0:QPC],
                        ut[:, pg, uoff + c * 128:uoff + c * 128 + pc],
                        rhs, start=(t == 0), stop=(t == NTERMS - 1))

            # ---- softmax weights: strided EXP over all 3 psT banks, split
            # per query half so qt0's output matmuls overlap qt1's EXP.
            for qt in range(2):
                qs = slice(qt * P, (qt + 1) * P)
                nc.scalar.activation(expw[:, :, qs], pT[:, :, qs], EXP,
                                     bias=z1, scale=1.0)

            # ---- output: out[q,d] = sum_k expw[k,q] hv[k,d] --------------
            for qt in range(2):
                qs = slice(qt * P, (qt + 1) * P)
                for c in range(KC):
                    pc = KCHUNK[c]
                    nc.tensor.matmul(ps_o[qt], expw[0:pc, c, qs],
                                     hv(c)[0:pc, :],
                                     start=(c == 0), stop=(c == KC - 1))
                nc.vector.reciprocal(rsum[qt], ps_o[qt][:, D:DV])
                nc.vector.tensor_scalar(out=osb[qt], in0=ps_o[qt][:, 0:D],
                                        scalar1=rsum[qt],
                                        scalar2=None, op0=ALU.mult)
                if qt == 0:
                    nc.sync.dma_start(out=out_ext[qs, :], in_=osb[qt])
                else:
                    nc.scalar.dma_start(out=out_ext[qs, :], in_=osb[qt])

    nc.compile()
    return nc


def _make_cst(Wt_f):
    cst = np.zeros((A, NCST), dtype=np.float32)
    for j in range(NB):
        cst[:, CST_S + j] = BETA[j] * Wt_f
    for d in range(ND):
        bd = BETA[NB + d]
        cst[:, CST_N + d] = -4.0 * bd * Wt_f
        cst[:, CST_A + d] = 2.0 * bd * Wt_f
    return cst


def _chunk_rows(a, nrow):
    """[nrow*128, W] -> per-partition packed [128, nrow*W]."""
    W = a.shape[1]
    return np.ascontiguousarray(
        a.reshape(nrow, 128, W).transpose(1, 0, 2).reshape(128, nrow * W))


def _pack_core(hq, htk, hv_pad, WuT16, WvT16):
    cols = [
        _chunk_rows(WvT16, 2),             # [128, 256]
        _chunk_rows(hq.T, 2),              # [128, 512]
        _chunk_rows(WuT16, 2),             # [128, 256]
        _chunk_rows(htk, 2),               # [128, 544]
        _chunk_rows(hv_pad, 3),            # [128, 3*DV]
    ]
    return np.ascontiguousarray(np.concatenate(cols, axis=1))


def kernel(hidden, mask, Wu, bu, Wv, Wt, bt):
    global _NC, LAST_EXEC_NS
    if _NC is None:
        _NC = _build_program()
    nc = _NC

    hidden = np.asarray(hidden, dtype=np.float32)
    mask = np.asarray(mask)
    Wu = np.asarray(Wu, dtype=np.float32)
    Wv = np.asarray(Wv, dtype=np.float32)
    Wt_f = np.asarray(Wt, dtype=np.float32).reshape(A)
    bu_f = np.asarray(bu, dtype=np.float32).reshape(A)

    WuT16 = Wu.T.astype(np.float16)        # [D, A]
    WvT16 = Wv.T.astype(np.float16)
    cst = _make_cst(Wt_f)
    bu16 = np.ascontiguousarray(bu_f.astype(np.float16).reshape(1, A))

    # per-batch gathered keys (shared by the two cores of a batch)
    batch_prep = []
    for b in range(B):
        valid = np.where(np.asarray(mask[b]) >= 1)[0]
        nv = len(valid)
        assert nv <= KW, f"valid keys {nv} > KW={KW}"
        hk = hidden[b][valid].astype(np.float16)            # [nv, D]
        htk = np.zeros((D, KW), dtype=np.float16)
        htk[:, :nv] = hk.T
        hv_pad = np.zeros((3 * 128, DV), dtype=np.float16)
        hv_pad[:nv, :D] = hk
        hv_pad[:nv, D] = 1.0               # ones only on VALID rows
        batch_prep.append((htk, hv_pad))

    in_maps = []
    for c in range(NCORES):
        b, half = divmod(c, 2)
        qoff = half * QPC
        htk, hv_pad = batch_prep[b]
        hq = hidden[b, qoff:qoff + QPC].astype(np.float16)
        pk = _pack_core(hq, htk, hv_pad, WuT16, WvT16)
        in_maps.append({"pk": pk, "bu16": bu16, "cst": cst})

    if TRACE:
        _ensure_ntff_hook()
    res = run_bass_kernel_spmd(nc, in_maps, list(range(NCORES)), trace=TRACE)
    LAST_EXEC_NS = res.exec_time_ns

    out = np.empty((B, S, D), dtype=np.float32)
    for c in range(NCORES):
        b, half = divmod(c, 2)
        qoff = half * QPC
        out[c // 2, qoff:qoff + QPC] = res.results[c]["out"].astype(np.float32)
    return out


# revision 37
# speedup vs baseline: 1.0508x; 1.0458x over previous
"""Additive (Bahdanau-style) attention on 8 Trainium2 NeuronCores.

Math: scores[b,q,k] = Wt . tanh(u[b,k] + v[b,q]) + bt, masked softmax over k,
out = weights @ hidden.  (bt dropped: softmax is shift-invariant.)

tanh(x) on |x| <= 9.95 ~= sum_m beta_m sin(om_m x) with 5 frequencies:
3 free bases (base 0 direct-range, bases 1-2 FRAC range-reduced) plus 2
derived doubles (2*base0, 2*base1) whose feature maps are pointwise products
of the base maps (sin2 = 2 s c, cos2 = 1 - 2 s^2), built on DVE + GPSIMD.
The angle-addition identity turns the [Sq,Sk,A] tanh tensor into PE matmuls
contracting over A.  Additive q-only score terms are softmax-invariant and
dropped; the key-side bias bu is folded into the v psum via a bu x ones seed
matmul so all range reductions are bias-free.

Masking: keys are host-gathered (padded to KW=272); pad rows get zero values
AND a zero in the denominator ones-column, so no mask bias is needed at all.
Scores accumulate transposed (psT[k,q]) in three separate PSUM banks; one
strided EXP over all three banks produces the weights; the output matmul's
ones-column yields the softmax denominator for free.

Layout: all big f16 operands ship in one packed [128, 2339] staging tensor,
split into two DMAs (proj operands first, values second) so the projection
starts as early as possible.  Garbage "heater" matmuls at the head keep the
PE HAM busy so the 4/8 cold clock throttle lifts before the real matmuls.

Sharding: core c -> batch b = c//2, query half qoff = (c%2)*256 (pure SPMD).
"""

import numpy as np

import concourse.bass as bass
import concourse.tile as tile
from concourse import bacc, mybir
from concourse.bass_utils import run_bass_kernel_spmd

# ---- problem constants (hardcoded; kernel.py must be self-contained) -------
B, S, D, A = 4, 512, 256, 128
QPC = 256          # queries per core
NCORES = 8
KW = 272           # gathered-key width (max valid count 271, padded)
KC = 3             # key chunks: 128 + 128 + 16
KCHUNK = (128, 128, 16)
FW = KW + QPC      # fused v|u feature row width
PV = 0             # v cols [0:QPC]
PU = QPC           # u cols [QPC:FW]; proj writes split at psum col 512
MAGIC = float(1.5 * 2 ** 23)     # fp32 round-to-nearest magic constant
TWO_PI = float(2.0 * np.pi)
DV = D + 1                       # values get a ones column -> denominator

# ---- tanh fit: 3 bases (base0 direct) + 2 derived (2x of bases 0,1) --------
BASES = [0.2625, 0.5905163459766474, 1.7487101922680963]
DIDX = [0, 1]
BETA = [1.4285084569414555, 1.3320835468154293, 0.08774613425554048,
        -1.2112188424747714, 0.1634796761779873]
NB = len(BASES)
ND = len(DIDX)
NTERMS = 2 * (NB + ND)

# packed f16 staging layout: per-partition column offsets
PK_WVT = 0                       # [2, 128]  wvT
PK_HTQ = PK_WVT + 256            # [2, 256]  hTq
PK_WUT = PK_HTQ + 512            # [2, 128]  wuT
PK_HTK = PK_WUT + 256            # [2, 272]  hTk
PK_HV = PK_HTK + 544             # [3, 257]  gathered values | ones(valid)
PK_TOT = PK_HV + 3 * DV          # 2339 cols f16

# const-tensor column layout: [A, NCST] f32
CST_S = 0                        # cols CST_S+j: beta_j * Wt (base v-scale)
CST_N = CST_S + NB               # cols CST_N+d: -4 beta_{NB+d} * Wt
CST_A = CST_N + ND               # cols CST_A+d:  2 beta_{NB+d} * Wt
NCST = CST_A + ND

TRACE = False                    # test.py sets True for the profiled run
LAST_EXEC_NS = None


def _ensure_ntff_hook():
    """The agent image's `antenv` lacks `axon_hooks`, so the boot-time NTFF
    hook registration silently degrades.  Recreate it: install a stub module
    and wire it to the ctypes profiler in trn_agent_boot."""
    import sys, types
    if "antenv.axon_hooks" in sys.modules:
        return
    mod = types.ModuleType("antenv.axon_hooks")
    _h = [None]
    mod.set_axon_ntff_profile_hook = lambda h: _h.__setitem__(0, h)
    mod.get_axon_ntff_profile_hook = lambda: _h[0]
    import antenv
    sys.modules["antenv.axon_hooks"] = mod
    antenv.axon_hooks = mod
    try:
        from trn_agent_boot.trn_boot import _ntff_profile_via_ctypes
        mod.set_axon_ntff_profile_hook(
            _ntff_profile_via_ctypes("/opt/axon/libaxon_pjrt.so"))
    except Exception:
        pass


# ---- custom DVE op (baseline-proven): out = t - round(t), t = in0*s0 + s1 --
_FRAC_OP = None


def _frac_reference(in0, in1, s0, s1, imm2):
    f32 = np.float32
    t = (in0.astype(f32) * f32(s0) + f32(s1)).astype(f32)
    r = ((t + f32(imm2)).astype(f32) - f32(imm2)).astype(f32)
    return (t - r).astype(f32)


def _get_frac_op():
    global _FRAC_OP
    if _FRAC_OP is not None:
        return _FRAC_OP
    from concourse import dve_ops as dvo
    from concourse.dve_spec import C0, C1, C2, Spec, Src0, lower, _has_src1
    from concourse.dve_uop import DveOpSpec

    name = "FRAC_AFFINE_ATT"
    for op in dvo.OPS:
        if op.name == name:
            _FRAC_OP = op
            return op
    t = Src0 * C0 + C1
    spec = Spec(body=t - ((t + C2) - C2), reference=_frac_reference)
    op = dvo.DveOp(name, spec, subdim=False, uops_sha={})
    dvo.OPS.append(op)
    dvo.CUSTOM_DVE_SPECS[name] = spec
    dvo._SUB_OPCODE_FOR_NAME[name] = max(dvo._SUB_OPCODE_FOR_NAME.values()) + 1
    assert dvo._SUB_OPCODE_FOR_NAME[name] < 0x20
    for ver in ("v3", "v4"):
        compiled = DveOpSpec(
            name=name,
            opcode=dvo.get_dve_sub_opcode(name),
            uops=lower(spec, ver=ver),
            rd1_en=_has_src1(spec),
        )
        op.uops_sha[ver] = compiled.sha(ver)
    _FRAC_OP = op
    return op


_NC = None


def _build_program():
    frac = _get_frac_op()
    f32 = mybir.dt.float32
    f16 = mybir.dt.float16
    nc = bacc.Bacc("TRN2", target_bir_lowering=False, debug=False,
                   num_devices=NCORES)

    pk_ext = nc.dram_tensor("pk", [128, PK_TOT], f16, kind="ExternalInput").ap()
    bu_ext = nc.dram_tensor("bu16", [1, A], f16, kind="ExternalInput").ap()
    cst_ext = nc.dram_tensor("cst", [A, NCST], f32, kind="ExternalInput").ap()
    out_ext = nc.dram_tensor("out", [QPC, D], f16, kind="ExternalOutput").ap()

    P = 128
    SIN = mybir.ActivationFunctionType.Sin
    EXP = mybir.ActivationFunctionType.Exp
    ALU = mybir.AluOpType

    with tile.TileContext(nc) as tc:
        import contextlib
        with contextlib.ExitStack() as ctx:
            const = ctx.enter_context(tc.tile_pool(name="const", bufs=1))
            pp_proj = ctx.enter_context(
                tc.tile_pool(name="pp_proj", bufs=1, space="PSUM"))
            pp_sc = ctx.enter_context(
                tc.tile_pool(name="pp_sc", bufs=1, space="PSUM"))
            pp_out = ctx.enter_context(
                tc.tile_pool(name="pp_out", bufs=1, space="PSUM"))

            # ---- SBUF tiles (all live whole-kernel) ----
            pk = const.tile([P, PK_TOT], f16, name="pk")
            cst = const.tile([P, NCST], f32, name="cst")
            bu_sb = const.tile([1, A], f16, name="bu")
            ones_q = const.tile([1, QPC], f16, name="onesq")
            hs = const.tile([P, 512], f16, name="hs")
            z1 = const.tile([P, 1], f32, name="z1")
            pio2t = const.tile([P, 1], f32, name="pio2")
            scr = const.tile([P, 1], f16, name="scr")
            ft = [const.tile([P, 2, FW], f16, name=f"ft{j}")
                  for j in range(NB)]
            fr = {j: const.tile([P, 2, FW], f16, name=f"fr{j}")
                  for j in (1, 2)}
            vmc = [const.tile([P, QPC], f16, name=f"vmc{j}")
                   for j in range(NB)]
            vms = [const.tile([P, QPC], f16, name=f"vms{j}")
                   for j in range(NB)]
            du = [const.tile([P, 2, KW], f16, name=f"du{d}")
                  for d in range(ND)]
            dp0 = [const.tile([P, QPC], f16, name=f"dp0_{d}")
                   for d in range(ND)]
            dp1 = [const.tile([P, QPC], f16, name=f"dp1_{d}")
                   for d in range(ND)]
            dva = [const.tile([P, QPC], f16, name=f"dva{d}")
                   for d in range(ND)]
            dvb = [const.tile([P, QPC], f16, name=f"dvb{d}")
                   for d in range(ND)]
            expw = const.tile([P, KC, QPC], f16, name="expw")
            osb = [const.tile([P, D], f16, name=f"osb{qt}") for qt in range(2)]
            rsum = [const.tile([P, 1], f32, name=f"rs{qt}") for qt in range(2)]

            # ---- PSUM tiles ----
            # v and u projections in SEPARATE banks so ACT (direct SINs)
            # and DVE (FRACs) can read them concurrently.
            pv = pp_proj.tile([P, QPC], f32, name="pv")            # bank 0
            pu = pp_proj.tile([P, KW], f32, name="pu")             # bank 1
            pT = pp_sc.tile([P, KC, 512], f32, name="pT")          # banks 2-4
            heat_ps = pp_out.tile([P, 512], f32, name="heat")      # bank 5
            ps_o = [pp_out.tile([P, DV], f32, name=f"ps_o{qt}")
                    for qt in range(2)]                            # banks 6-7

            def wvT(c):
                return pk[:, PK_WVT + c * 128:PK_WVT + (c + 1) * 128]

            def hTq(c):
                return pk[:, PK_HTQ + c * 256:PK_HTQ + (c + 1) * 256]

            def wuT(c):
                return pk[:, PK_WUT + c * 128:PK_WUT + (c + 1) * 128]

            def hTk(c):
                return pk[:, PK_HTK + c * KW:PK_HTK + (c + 1) * KW]

            def hv(c):
                return pk[:, PK_HV + c * DV:PK_HV + (c + 1) * DV]

            # ---- early constants (gpsimd queue; DVE queue stays clear) ----
            nc.gpsimd.memset(z1, 0.0)
            nc.gpsimd.memset(pio2t, float(np.pi / 2))
            nc.gpsimd.memset(ones_q, 1.0)
            nc.vector.memset(hs, 0.5)

            # ---- input DMAs: one big-descriptor transfer for proj data ----
            nc.sync.dma_start(out=pk[:, 0:PK_HV], in_=pk_ext[:, 0:PK_HV])
            nc.sync.dma_start(out=pk[:, PK_HV:PK_TOT],
                              in_=pk_ext[:, PK_HV:PK_TOT])
            nc.scalar.dma_start(out=bu_sb, in_=bu_ext[:])
            nc.scalar.dma_start(out=cst, in_=cst_ext[:])

            # warm the trig table immediately (z1 is memset, not DMA-gated)
            nc.scalar.activation(scr, z1, SIN, bias=z1, scale=1.0)

            # ---- PE clock heaters: garbage matmuls lift the 4/8 throttle --
            def heat(n, width=512):
                for _ in range(n):
                    nc.tensor.matmul(heat_ps[:, 0:width], hs[:, 0:P],
                                     hs[:, 0:width], start=True, stop=True,
                                     skip_group_check=True)

            heat(6)

            # ---- projections: v-group -> pv (bank 0), u-group -> pu ----
            nc.tensor.matmul(pv, wvT(0), hTq(0), start=True, stop=False)
            nc.tensor.matmul(pv, wvT(1), hTq(1), start=False, stop=False)
            nc.tensor.matmul(pv, bu_sb, ones_q, start=False, stop=True)
            for c in range(2):
                nc.tensor.matmul(pu, wuT(c), hTk(c),
                                 start=(c == 0), stop=(c == 1))

            heat(5)

            # ---- range reductions (DVE) read pv/pu banks directly; the
            # direct-base SINs (ACT) interleave on the opposite bank.
            def emit_frac_u(j):
                s0 = float(BASES[j] / TWO_PI)
                nc.vector._custom_dve(frac, out=fr[j][:, 0, PU:FW], in0=pu,
                                      s0=s0, s1=0.0, imm2=MAGIC)
                nc.vector._custom_dve(frac, out=fr[j][:, 1, PU:FW], in0=pu,
                                      s0=s0, s1=0.25, imm2=MAGIC)

            def emit_frac_v(j):
                s0 = float(BASES[j] / TWO_PI)
                nc.vector._custom_dve(frac, out=fr[j][:, 0, PV:PV + QPC],
                                      in0=pv, s0=s0, s1=0.0, imm2=MAGIC)
                nc.vector._custom_dve(frac, out=fr[j][:, 1, PV:PV + QPC],
                                      in0=pv, s0=s0, s1=0.25, imm2=MAGIC)

            # ---- direct-base sines straight from psum (ACT), v then u ----
            w0 = float(BASES[0])
            nc.scalar.activation(ft[0][:, 0, PV:PV + QPC], pv, SIN,
                                 bias=z1, scale=w0)
            nc.scalar.activation(ft[0][:, 1, PV:PV + QPC], pv, SIN,
                                 bias=pio2t, scale=w0)

            # ---- v-side scales + derived maps ----
            def emit_vscale(j, eng=None):
                eng = eng or nc.vector
                sc_col = cst[:, CST_S + j:CST_S + j + 1]
                eng.tensor_scalar(
                    out=vmc[j], in0=ft[j][:, 1, PV:PV + QPC],
                    scalar1=sc_col, scalar2=None, op0=ALU.mult)
                eng.tensor_scalar(
                    out=vms[j], in0=ft[j][:, 0, PV:PV + QPC],
                    scalar1=sc_col, scalar2=None, op0=ALU.mult)

            def emit_derived_u(d):
                # GPSIMD: du pages (s_u^2, s_u c_u) from parent maps
                p = DIDX[d]
                su = ft[p][:, 0, PU:FW]
                cu = ft[p][:, 1, PU:FW]
                nc.gpsimd.tensor_tensor(out=du[d][:, 0, :], in0=su, in1=su,
                                        op=ALU.mult)
                nc.gpsimd.tensor_tensor(out=du[d][:, 1, :], in0=su, in1=cu,
                                        op=ALU.mult)

            def emit_derived_v(d):
                # dp0 = S_p*Wt*sv^2, dp1 = S_p*Wt*sv*cv (plain TT from vms);
                # va = cdiv*dp0 + 2 b_d Wt;  vb = cdiv*dp1,  cdiv=-4 b_d/b_p
                p = DIDX[d]
                sv = ft[p][:, 0, PV:PV + QPC]
                cv = ft[p][:, 1, PV:PV + QPC]
                cdiv = float(-4.0 * BETA[NB + d] / BETA[p])
                aCol = cst[:, CST_A + d:CST_A + d + 1]
                nc.vector.tensor_tensor(out=dp0[d], in0=vms[p], in1=sv,
                                        op=ALU.mult)
                nc.vector.tensor_tensor(out=dp1[d], in0=vms[p], in1=cv,
                                        op=ALU.mult)
                nc.vector.tensor_scalar(
                    out=dva[d], in0=dp0[d], scalar1=cdiv, scalar2=None,
                    op0=ALU.mult)
                nc.vector.tensor_scalar(
                    out=dva[d], in0=dva[d], scalar1=aCol, scalar2=None,
                    op0=ALU.add)
                nc.vector.tensor_scalar(
                    out=dvb[d], in0=dp1[d], scalar1=cdiv, scalar2=None,
                    op0=ALU.mult)

            emit_frac_u(1)
            nc.scalar.activation(ft[0][:, 0, PU:FW], pu, SIN,
                                 bias=z1, scale=w0)
            nc.scalar.activation(ft[0][:, 1, PU:FW], pu, SIN,
                                 bias=pio2t, scale=w0)
            emit_vscale(0)
            emit_frac_v(1)
            nc.scalar.activation(ft[1], fr[1], SIN, bias=z1, scale=TWO_PI)
            emit_derived_v(0)
            emit_derived_u(0)
            emit_frac_u(2)
            emit_frac_v(2)
            nc.scalar.activation(ft[2], fr[2], SIN, bias=z1, scale=TWO_PI)
            emit_vscale(1)
            emit_derived_u(1)
            emit_derived_v(1)
            emit_vscale(2)

            # ---- score matmuls, transposed psT[k,q], term-major ----------
            # term list in production order: (u_tile, page, rhs)
            terms = [
                (ft[0], 0, vmc[0]), (ft[0], 1, vms[0]),
                (du[0], 1, dva[0]), (du[0], 0, dvb[0]),
                (ft[1], 0, vmc[1]), (ft[1], 1, vms[1]),
                (du[1], 1, dva[1]), (du[1], 0, dvb[1]),
                (ft[2], 0, vmc[2]), (ft[2], 1, vms[2]),
            ]
            assert len(terms) == NTERMS
            for t, (ut, pg, rhs) in enumerate(terms):
                uoff = PU if ut.shape[2] == FW else 0
                for c in range(KC):
                    pc = KCHUNK[c]
                    nc.tensor.matmul(
                        pT[0:pc, c, 0:QPC],
                        ut[:, pg, uoff + c * 128:uoff + c * 128 + pc],
                        rhs, start=(t == 0), stop=(t == NTERMS - 1))

            # ---- softmax weights: strided EXP over all 3 psT banks, split
            # per query half so qt0's output matmuls overlap qt1's EXP.
            for qt in range(2):
                qs = slice(qt * P, (qt + 1) * P)
                nc.scalar.activation(expw[:, :, qs], pT[:, :, qs], EXP,
                                     bias=z1, scale=1.0)

            # ---- output: out[q,d] = sum_k expw[k,q] hv[k,d] --------------
            for qt in range(2):
                qs = slice(qt * P, (qt + 1) * P)
                for c in range(KC):
                    pc = KCHUNK[c]
                    nc.tensor.matmul(ps_o[qt], expw[0:pc, c, qs],
                                     hv(c)[0:pc, :],
                                     start=(c == 0), stop=(c == KC - 1))
                nc.vector.reciprocal(rsum[qt], ps_o[qt][:, D:DV])
                nc.vector.tensor_scalar(out=osb[qt], in0=ps_o[qt][:, 0:D],
                                        scalar1=rsum[qt],
                                        scalar2=None, op0=ALU.mult)
                if qt == 0:
                    nc.sync.dma_start(out=out_ext[qs, :], in_=osb[qt])
                else:
                    nc.scalar.dma_start(out=out_ext[qs, :], in_=osb[qt])

    nc.compile()
    return nc


def _make_cst(Wt_f):
    cst = np.zeros((A, NCST), dtype=np.float32)
    for j in range(NB):
        cst[:, CST_S + j] = BETA[j] * Wt_f
    for d in range(ND):
        bd = BETA[NB + d]
        cst[:, CST_N + d] = -4.0 * bd * Wt_f
        cst[:, CST_A + d] = 2.0 * bd * Wt_f
    return cst


def _chunk_rows(a, nrow):
    """[nrow*128, W] -> per-partition packed [128, nrow*W]."""
    W = a.shape[1]
    return np.ascontiguousarray(
        a.reshape(nrow, 128, W).transpose(1, 0, 2).reshape(128, nrow * W))


def _pack_core(hq, htk, hv_pad, WuT16, WvT16):
    cols = [
        _chunk_rows(WvT16, 2),             # [128, 256]
        _chunk_rows(hq.T, 2),              # [128, 512]
        _chunk_rows(WuT16, 2),             # [128, 256]
        _chunk_rows(htk, 2),               # [128, 544]
        _chunk_rows(hv_pad, 3),            # [128, 3*DV]
    ]
    return np.ascontiguousarray(np.concatenate(cols, axis=1))


def kernel(hidden, mask, Wu, bu, Wv, Wt, bt):
    global _NC, LAST_EXEC_NS
    if _NC is None:
        _NC = _build_program()
    nc = _NC

    hidden = np.asarray(hidden, dtype=np.float32)
    mask = np.asarray(mask)
    Wu = np.asarray(Wu, dtype=np.float32)
    Wv = np.asarray(Wv, dtype=np.float32)
    Wt_f = np.asarray(Wt, dtype=np.float32).reshape(A)
    bu_f = np.asarray(bu, dtype=np.float32).reshape(A)

    WuT16 = Wu.T.astype(np.float16)        # [D, A]
    WvT16 = Wv.T.astype(np.float16)
    cst = _make_cst(Wt_f)
    bu16 = np.ascontiguousarray(bu_f.astype(np.float16).reshape(1, A))

    # per-batch gathered keys (shared by the two cores of a batch)
    batch_prep = []
    for b in range(B):
        valid = np.where(np.asarray(mask[b]) >= 1)[0]
        nv = len(valid)
        assert nv <= KW, f"valid keys {nv} > KW={KW}"
        hk = hidden[b][valid].astype(np.float16)            # [nv, D]
        htk = np.zeros((D, KW), dtype=np.float16)
        htk[:, :nv] = hk.T
        hv_pad = np.zeros((3 * 128, DV), dtype=np.float16)
        hv_pad[:nv, :D] = hk
        hv_pad[:nv, D] = 1.0               # ones only on VALID rows
        batch_prep.append((htk, hv_pad))

    in_maps = []
    for c in range(NCORES):
        b, half = divmod(c, 2)
        qoff = half * QPC
        htk, hv_pad = batch_prep[b]
        hq = hidden[b, qoff:qoff + QPC].astype(np.float16)
        pk = _pack_core(hq, htk, hv_pad, WuT16, WvT16)
        in_maps.append({"pk": pk, "bu16": bu16, "cst": cst})

    if TRACE:
        _ensure_ntff_hook()
    res = run_bass_kernel_spmd(nc, in_maps, list(range(NCORES)), trace=TRACE)
    LAST_EXEC_NS = res.exec_time_ns

    out = np.empty((B, S, D), dtype=np.float32)
    for c in range(NCORES):
        b, half = divmod(c, 2)
        qoff = half * QPC
        out[c // 2, qoff:qoff + QPC] = res.results[c]["out"].astype(np.float32)
    return out
